# revision 35
# baseline (speedup 1.0000x reference)
"""AnomalyScores (PatchCore-style 1-NN retrieval) Trainium2 kernel.

Sharding: data-parallel over the batch dim — core i owns batch i's 784
patches; the 16384x384 coreset is replicated on every core. All compute
is core-local; no collectives.

v2 design (fp8 DoubleRow, transposed main pass):
  * u = b2' - 2ab computed with fp8e4 DoubleRow matmuls (K padded to 512,
    2 chunks of 256; 2 PE instructions per output tile vs bf16's 3, at
    half the per-row cost). b2' rides along as two extra contraction rows
    (fp8 hi+lo) against constant-1 rows on the E side, so PSUM holds u
    directly. a2 is computed on the host.
  * TRANSPOSED orientation: coreset points on partitions (always 128/128
    full), patches along free. PSUM tile [128, 4, 392] = 4 n-blocks x one
    392-patch slice. Per-patch min = fold over tiles (elementwise) + a
    final Pool C-axis reduce over partitions, which lands srow directly.
  * PSUM drain split across engines (only DVE/ACT may read PSUM):
      D-route : DVE scalar_tensor_tensor min straight off PSUM into a wide
                bf16 accumulator (1 elem/cycle).
      AD-route: ACT copies the tile to bf16 SBUF; DVE folds at 4x DVE mode.
      AP-route: ACT copies; the Pool engine folds (SBUF-only).
  * Tail (patch argmax, 1-NN re-query via matvecs, top-9 support, softmax
    weight) follows the v1 structure, adapted to the fp8 layout.
Output per core: one f32 scalar; host concatenates 8 cores -> [8].
"""

import sys

import numpy as np
import ml_dtypes

if "/opt/trn_rl_repo" not in sys.path:
    sys.path.insert(0, "/opt/trn_rl_repo")

import concourse.bass as bass
import concourse.mybir as mybir
import concourse.tile as tile
from concourse import bacc, bass_isa
from concourse.bass import ds
from concourse.masks import make_identity
from concourse.bass_utils import run_bass_kernel_spmd

FP8 = ml_dtypes.float8_e4m3
F32 = mybir.dt.float32
F8 = mybir.dt.float8e4
BF = mybir.dt.bfloat16
U32 = mybir.dt.uint32

B, P, D, N = 8, 784, 384, 16384
KJ = 2            # DoubleRow K chunks of 256 (K padded 384+2 -> 512)
NBG = 32          # groups of 4 n-blocks (128 n-blocks of 128 points)
SL = 392          # patch slice width (2 slices of 392)
KB = 8            # AC-route partials staged per scatter DMA
C0 = 384.0        # b2 centering constant (E[|c|^2] = D)
BIG = 3.0e38

# drain route mix per (nb-pair, slice) tile: DVE-direct / ACT+Pool
ROUTE_WEIGHTS = {"D": 59, "AC": 69}

Alu = mybir.AluOpType
Act = mybir.ActivationFunctionType
Axis = mybir.AxisListType
DR = mybir.MatmulPerfMode.DoubleRow


def _route_schedule(n):
    # deterministic weighted round-robin (error diffusion)
    w = dict(ROUTE_WEIGHTS)
    tot = sum(w.values())
    cred = {k: 0.0 for k in w}
    out = []
    for _ in range(n):
        for k in w:
            cred[k] += w[k] / tot
        k = max(cred, key=lambda q: cred[q])
        cred[k] -= 1.0
        out.append(k)
    # end with D tiles so the final AC stage-flush DMA overlaps compute
    for i in range(n - 6, n):
        if out[i] != "D":
            j = max(x for x in range(i) if out[x] == "D")
            out[i], out[j] = out[j], out[i]
    return out


def _build(stage=99):
    nc = _build_inner(stage)
    nc.finalize()
    return nc


def _build_inner(stage=99):
    nc = bacc.Bacc("TRN2", target_bir_lowering=False, debug=False)

    ct_d = [
        nc.dram_tensor(f"ct{j}", [128, 2, N], F8, kind="ExternalInput")
        for j in range(KJ)
    ]
    emt_d = [
        nc.dram_tensor(f"emt{j}", [128, 2, P], F8, kind="ExternalInput")
        for j in range(KJ)
    ]
    a2row_d = nc.dram_tensor("a2row", [1, P], F32, kind="ExternalInput")
    out_d = nc.dram_tensor("out", [1], F32, kind="ExternalOutput")

    with tile.TileContext(nc) as tc:
        with (
            tc.tile_pool(name="constp", bufs=1) as constp,
            tc.tile_pool(name="workp", bufs=2) as workp,
            tc.tile_pool(name="psump", bufs=4, space="PSUM") as psump,
            tc.tile_pool(name="bouncep", bufs=4) as bouncep,
            tc.tile_pool(name="dramp", bufs=1, space="DRAM") as dramp,
        ):
            # ---------------- resident inputs ----------------
            emt_sb = []
            for j in range(KJ):
                t = constp.tile([128, 2, P], F8, name=f"emt_sb{j}")
                nc.sync.dma_start(out=t, in_=emt_d[j][:, :, :])
                emt_sb.append(t)
            ct_sb = [
                constp.tile([128, 2, N], F8, name=f"ct_sb{j}") for j in range(KJ)
            ]
            for g in range(8):
                lo, hi = g * 2048, (g + 1) * 2048
                for j in range(KJ):
                    nc.sync.dma_start(
                        out=ct_sb[j][:, :, lo:hi], in_=ct_d[j][:, :, lo:hi]
                    )
            a2sb = constp.tile([1, P], F32, name="a2sb")
            nc.sync.dma_start(out=a2sb, in_=a2row_d[0:1, :])

            warm = constp.tile([1, 1], F32, name="warm")
            nc.vector.memset(warm, 1.0)
            # preload sqrt_and_friends (has Copy too -> one load for the
            # whole main loop + Sqrt tail)
            nc.scalar.activation(out=warm, in_=warm, func=Act.Sqrt)

            ones_f = constp.tile([1, 128], F32, name="ones_f")
            nc.vector.memset(ones_f, 1.0)
            pidx1 = constp.tile([128, 1], U32, name="pidx1")
            nc.gpsimd.iota(pidx1, pattern=[[0, 1]], base=0, channel_multiplier=1)
            ident = constp.tile([128, 128], F32, name="ident")
            make_identity(nc, ident)

            srow = constp.tile([1, P], F32, name="srow")

            # ---------------- main distance pass (transposed) ----------------
            # per (column-group g, patch-slice s) tile (2 n-blocks x SL
            # patches in 2 PSUM banks), one of two drain routes:
            #   D : DVE scalar_tensor_tensor (negate+max) off PSUM into the
            #       per-slice wide bf16 accumulator accD[s].
            #   AC: ACT negate-copies to bf16; Pool C-reduces the 128
            #       partitions to a [1, 2*SL] partial; an SBUF->SBUF DMA
            #       scatters it onto its own partition row of pgrid[s]
            #       (DMA engines + SP are otherwise idle); one final Pool
            #       C-reduce collapses all rows.
            accD = [constp.tile([128, 2, SL], BF, name=f"accD{s}") for s in range(2)]
            pgrid = [
                constp.tile([128, 2, SL], BF, name=f"pgrid{s}") for s in range(2)
            ]
            for s in range(2):
                nc.gpsimd.memset(accD[s], -BIG)
                nc.gpsimd.memset(pgrid[s], -BIG)
            sched = _route_schedule(NBG * 4)
            ti = 0
            prow = [0, 0]
            stg = [None, None]
            for g in range(NBG * 2):
                for s in range(2):
                    ps = psump.tile([128, 2, 512], F32, name="ps", tag="ps")
                    for q in range(2):
                        nb = g * 2 + q
                        for j in range(KJ):
                            nc.tensor.matmul(
                                ps[:, q, 0:SL],
                                lhsT=ct_sb[j][:, :, nb * 128 : (nb + 1) * 128],
                                rhs=emt_sb[j][:, :, s * SL : (s + 1) * SL],
                                start=(j == 0), stop=(j == KJ - 1),
                                perf_mode=DR,
                            )
                    r = sched[ti]
                    ti += 1
                    if r == "D":
                        nc.vector.scalar_tensor_tensor(
                            out=accD[s], in0=ps[:, :, 0:SL], scalar=-1.0,
                            in1=accD[s], op0=Alu.mult, op1=Alu.max,
                        )
                    else:
                        bfc = workp.tile([128, 2, SL], BF, name="bfc", tag="bfp")
                        nc.scalar.mul(bfc, ps[:, :, 0:SL], -1.0)
                        t = prow[s]
                        prow[s] += 1
                        si, sk = divmod(t, KB)
                        if sk == 0:
                            stg[s] = bouncep.tile(
                                [1, KB, 2, SL], BF, name="stage", tag=f"st{s}"
                            )
                        nc.gpsimd.tensor_reduce(
                            out=stg[s][0:1, sk, :, :], in_=bfc,
                            axis=Axis.C, op=Alu.max,
                        )
                        if sk == KB - 1:
                            nc.sync.dma_start(
                                out=pgrid[s][si * KB : (si + 1) * KB, :, :],
                                in_=stg[s].rearrange("o k a b -> (o k) a b"),
                            )

            for s in range(2):
                si, sk = divmod(prow[s], KB)
                if sk:
                    nc.sync.dma_start(
                        out=pgrid[s][si * KB : si * KB + sk, :, :],
                        in_=stg[s][0:1, 0:sk, :, :].rearrange(
                            "o k a b -> (o k) a b"
                        ),
                    )
            # finals: per slice, C-reduce pgrid and accD, DVE-merge the two
            # partials, fold lanes, score^2 = a2 - nu_max -> srow
            srow2 = constp.tile([1, 2, 2, 2, SL], F32, name="srow2")
            for s in range(2):
                nc.gpsimd.tensor_reduce(
                    out=srow2[0:1, s, 0, :, :], in_=pgrid[s],
                    axis=Axis.C, op=Alu.max,
                )
                nc.gpsimd.tensor_reduce(
                    out=srow2[0:1, s, 1, :, :], in_=accD[s],
                    axis=Axis.C, op=Alu.max,
                )
            for s in range(2):
                nc.vector.tensor_tensor(
                    out=srow2[0:1, s, 0, :, :], in0=srow2[0:1, s, 1, :, :],
                    in1=srow2[0:1, s, 0, :, :], op=Alu.max,
                )
                nc.vector.tensor_tensor(
                    out=srow2[0:1, s, 0, 0, :], in0=srow2[0:1, s, 0, 1, :],
                    in1=srow2[0:1, s, 0, 0, :], op=Alu.max,
                )
                # score^2 = a2 - nu_max; sqrt deferred (monotone): the patch
                # argmax runs on score^2; only the winner's sqrt is needed.
                nc.vector.tensor_sub(
                    srow[0:1, s * SL : (s + 1) * SL],
                    a2sb[0:1, s * SL : (s + 1) * SL],
                    srow2[0:1, s, 0, 0, :],
                )

            # ---------------- patch argmax (on score^2) ----------------
            s8 = constp.tile([1, 8], F32, name="s8")
            sidx8 = constp.tile([1, 8], U32, name="sidx8")
            nc.vector.max_with_indices(s8, sidx8, srow)
            scoresc = constp.tile([1, 1], F32, name="scoresc")
            nc.scalar.activation(out=scoresc, in_=s8[0:1, 0:1], func=Act.Sqrt)
            if stage <= 0:
                nc.sync.dma_start(out=out_d[:], in_=s8[0:1, 0:1])
                return nc
            mp_reg = nc.values_load(
                sidx8[0:1, 0:1],
                engines=[mybir.EngineType.DVE, mybir.EngineType.SP],
                min_val=0, max_val=P - 1, skip_runtime_bounds_check=True,
            )

            # ---------------- part A: u-row of distances from E[mp] ----------------
            ecol = []
            for j in range(KJ):
                c = constp.tile([128, 2, 1], F8, name=f"ecol{j}")
                nc.vector.tensor_copy(c, emt_sb[j][:, :, ds(mp_reg, 1)])
                ecol.append(c)
            if stage <= 1:
                nc.sync.dma_start(out=out_d[:], in_=s8[0:1, 0:1])
                return nc

            psA = psump.tile([128, 2, 512], F32, name="psA", tag="ps")
            psA_v = psA[:, 0]
            for c in range(128):
                for j in range(KJ):
                    nc.tensor.matmul(
                        psA_v[:, c : c + 1],
                        lhsT=ct_sb[j][:, :, c * 128 : (c + 1) * 128],
                        rhs=ecol[j],
                        start=(j == 0),
                        stop=(j == KJ - 1),
                        perf_mode=DR,
                    )
            nu_sb = constp.tile([128, 128], F32, name="nu_sb")
            nc.vector.tensor_scalar_mul(nu_sb, psA_v[:, 0:128], -1.0)

            # cross-partition argmax via PE transpose (no DRAM bounces):
            # va2 = [per-partition max nu | grid index as f32] -> [2, 128]
            vals8 = constp.tile([128, 8], F32, name="vals8")
            idx8 = constp.tile([128, 8], U32, name="idx8")
            nc.vector.max_with_indices(vals8, idx8, nu_sb)
            gidxn = constp.tile([128, 1], U32, name="gidxn")
            nc.vector.tensor_scalar_mul(gidxn, idx8[:, 0:1], 128)
            nc.vector.tensor_add(gidxn, gidxn, pidx1)
            va2 = constp.tile([128, 2], F32, name="va2")
            nc.vector.tensor_copy(va2[:, 0:1], vals8[:, 0:1])
            nc.vector.tensor_copy(va2[:, 1:2], gidxn)  # u32 -> f32 (exact)
            pst = psump.tile([128, 2, 512], F32, name="pst", tag="ps")
            nc.tensor.transpose(pst[0:2, 0, 0:128], va2, ident)
            trow = constp.tile([2, 128], F32, name="trow")
            nc.vector.tensor_copy(trow, pst[0:2, 0, 0:128])
            m8 = constp.tile([1, 8], F32, name="m8")
            mi8 = constp.tile([1, 8], U32, name="mi8")
            nc.vector.max_with_indices(m8, mi8, trow[0:1, :])
            pstar_reg = nc.values_load(
                mi8[0:1, 0:1], engines=[mybir.EngineType.DVE],
                min_val=0, max_val=127, skip_runtime_bounds_check=True,
            )
            nn32 = constp.tile([2, 1], U32, name="nn32")
            nc.vector.tensor_copy(nn32, trow[0:2, ds(pstar_reg, 1)])
            nn_reg = nc.values_load(
                nn32[1:2, 0:1], engines=[mybir.EngineType.DVE],
                min_val=0, max_val=N - 1, skip_runtime_bounds_check=True,
            )
            # d_grid = sqrt(bias - nu) : the row's true distances (bias = a2+C0
            # of the max patch, picked straight out of SBUF).
            bias1 = constp.tile([1, 1], F32, name="bias1")
            nc.vector.tensor_copy(bias1, a2sb[0:1, ds(mp_reg, 1)])
            psb = psump.tile([128, 2, 512], F32, name="psb", tag="ps")
            nc.tensor.matmul(
                psb[:, 0, 0:1], lhsT=ones_f, rhs=bias1, start=True, stop=True
            )
            bias_col = constp.tile([128, 1], F32, name="bias_col")
            nc.vector.tensor_copy(bias_col, psb[:, 0, 0:1])
            dgrid = constp.tile([128, 128], F32, name="dgrid")
            nc.scalar.activation(
                out=dgrid, in_=nu_sb, func=Act.Sqrt, bias=bias_col, scale=-1.0
            )
            egrid = constp.tile([128, 128], F32, name="egrid")
            nc.scalar.activation(out=egrid, in_=dgrid, func=Act.Exp)
            # e0 = exp(score); the zero bias read from egrid pins it after
            # egrid so the scheduler cannot hoist it into the Sqrt phase
            ezero = constp.tile([1, 1], F32, name="ezero")
            nc.vector.tensor_scalar_mul(ezero, egrid[0:1, 0:1], 0.0)
            e0 = constp.tile([1, 1], F32, name="e0")
            nc.scalar.activation(out=e0, in_=scoresc, func=Act.Exp, bias=ezero)
            if stage <= 2:
                nc.sync.dma_start(out=out_d[:], in_=m8[0:1, 0:1])
                return nc

            # ---------------- part B: u-row from C[nn] + top-9 ----------------
            # ccol = -2 * (nn column of ct); the b2h/b2l entries (j=1, i=1,
            # p=0/1) must be the constant 1 instead -> memset patch.
            ccol = []
            for j in range(KJ):
                c = constp.tile([128, 2, 1], F8, name=f"ccol{j}")
                nc.vector.tensor_scalar_mul(c, ct_sb[j][:, :, ds(nn_reg, 1)], -2.0)
                ccol.append(c)
            nc.vector.memset(ccol[1][0:2, 1, 0:1], 1.0)
            psB = psump.tile([128, 2, 512], F32, name="psB", tag="ps")
            psB_v = psB[:, 0]
            for c in range(128):
                for j in range(KJ):
                    nc.tensor.matmul(
                        psB_v[:, c : c + 1],
                        lhsT=ct_sb[j][:, :, c * 128 : (c + 1) * 128],
                        rhs=ccol[j],
                        start=(j == 0),
                        stop=(j == KJ - 1),
                        perf_mode=DR,
                    )
            nu2_sb = constp.tile([128, 128], F32, name="nu2_sb")
            nc.vector.tensor_scalar_mul(nu2_sb, psB_v[:, 0:128], -1.0)
            # the support set = 9 largest nu2 (= 9 smallest d_nn). Only the
            # 9th-largest VALUE is needed: it thresholds a mask over the grid,
            # and the softmax denominator is the masked sum of exp(dgrid).
            # top-4 per partition suffices for a global top-9 (a partition
            # holding >=5 of the 9 would be needed to break this; verified
            # exact on this dataset)
            # thr between the 9th and 10th largest nu2, exact over all
            # 16384 via the gpsimd kth_largest heap (quantile chosen so
            # k_adj = 8 with a 0.5 lerp toward desc[9])
            t8b = constp.tile([1, 2], F32, name="t8b")
            nc.gpsimd.kth_largest(
                t8b, nu2_sb, n_per_lane=128, k=10, quantile=1.0 - 8.5 / 16383.0
            )
            if stage <= 3:
                nc.sync.dma_start(out=out_d[:], in_=t8b[0:1, 0:1])
                return nc

            # broadcast thr to all partitions via PE
            psc = psump.tile([128, 2, 512], F32, name="psc", tag="ps")
            nc.tensor.matmul(
                psc[:, 0, 0:1], lhsT=ones_f, rhs=t8b[0:1, 1:2],
                start=True, stop=True,
            )
            thr_col = constp.tile([128, 1], F32, name="thr_col")
            nc.vector.tensor_copy(thr_col, psc[:, 0, 0:1])
            mask = constp.tile([128, 128], F32, name="mask")
            # thr = desc[10th]; strict > keeps exactly the top-9 (exact
            # element from the heap, no lerp dependence)
            nc.vector.tensor_scalar(
                mask, nu2_sb, thr_col, None, op0=Alu.is_gt
            )
            msum = constp.tile([128, 1], F32, name="msum")
            nc.vector.tensor_mul(mask, mask, egrid)
            nc.vector.tensor_reduce(out=msum, in_=mask, axis=Axis.X, op=Alu.add)
            msum1 = constp.tile([1, 1], F32, name="msum1")
            nc.gpsimd.tensor_reduce(
                out=msum1, in_=msum, axis=Axis.C, op=Alu.add
            )

            # ---------------- softmax weight ----------------
            # weight = 1 - exp(score)/sum; out = weight * score
            sinv = constp.tile([1, 1], F32, name="sinv")
            nc.vector.reciprocal(sinv, msum1)
            p0 = constp.tile([1, 1], F32, name="p0")
            nc.vector.tensor_mul(p0, e0, sinv)
            w = constp.tile([1, 1], F32, name="w")
            nc.vector.tensor_scalar(w, p0, -1.0, 1.0, op0=Alu.mult, op1=Alu.add)
            outv = constp.tile([1, 1], F32, name="outv")
            nc.vector.tensor_mul(outv, w, scoresc)
            nc.sync.dma_start(out=out_d[:], in_=outv)

    return nc


_NC = None


def _get_nc():
    global _NC
    if _NC is None:
        import os

        _NC = _build(stage=int(os.environ.get("KSTAGE", "99")))
    return _NC


def _prep_inputs(embedding, embedding_coreset):
    E = np.ascontiguousarray(np.asarray(embedding, dtype=np.float32))
    C = np.ascontiguousarray(np.asarray(embedding_coreset, dtype=np.float32))
    b2 = np.sum(C.astype(np.float64) * C, axis=1).astype(np.float32)
    b2c = b2 - C0
    b2h = b2c.astype(FP8)
    b2l = (b2c - b2h.astype(np.float32)).astype(FP8)

    # Chat rows 0..383 = fp8(C)^T, 384 = b2h, 385 = b2l, 386..511 = 0
    chat = np.zeros((512, N), dtype=FP8)
    chat[0:D] = C.astype(FP8).T
    chat[D] = b2h
    chat[D + 1] = b2l
    ct = [
        np.ascontiguousarray(
            chat[256 * j : 256 * (j + 1)].reshape(2, 128, N).transpose(1, 0, 2)
        )
        for j in range(KJ)
    ]

    a2 = (np.sum(E.astype(np.float64) * E, axis=1) + C0).astype(np.float32)  # [B*P]

    in_maps = []
    for i in range(B):
        Eb = E[i * P : (i + 1) * P]
        ehat = np.zeros((512, P), dtype=FP8)
        ehat[0:D] = (-2.0 * Eb).astype(FP8).T
        ehat[D] = 1.0
        ehat[D + 1] = 1.0
        emt = [
            np.ascontiguousarray(
                ehat[256 * j : 256 * (j + 1)].reshape(2, 128, P).transpose(1, 0, 2)
            )
            for j in range(KJ)
        ]
        in_maps.append(
            {
                "ct0": ct[0],
                "ct1": ct[1],
                "emt0": emt[0],
                "emt1": emt[1],
                "a2row": np.ascontiguousarray(a2[i * P : (i + 1) * P][None, :]),
            }
        )
    return in_maps


def _run(embedding, embedding_coreset, batch_size, trace=False, **trace_kwargs):
    assert int(batch_size) == B
    in_maps = _prep_inputs(embedding, embedding_coreset)
    nc = _get_nc()
    res = run_bass_kernel_spmd(
        nc, in_maps, core_ids=list(range(B)), trace=trace, **trace_kwargs
    )
    out = np.array(
        [np.asarray(res.results[i]["out"]).reshape(-1)[0] for i in range(B)],
        dtype=np.float32,
    )
    return out, res


def kernel(embedding, embedding_coreset, batch_size):
    out, _ = _run(embedding, embedding_coreset, batch_size, trace=False)
    return out
